# revision 53
# baseline (speedup 1.0000x reference)
"""Axial (width+height) attention kernel for TRN2, 8 NeuronCores, SPMD.

Problem: nn_Attention_36507222016283
  hidden (128,128,1024) -> QKV proj (16 heads x 64) -> RoPE(cos,sin) ->
  width attention (per h-row) and height attention (per w-col), both with
  scale 1/sqrt(1024) and zero mask -> concat -> out proj Wo (2048->1024).

Sharding (head-parallel, zero-collective): core i owns heads {2i, 2i+1}.
Each core computes q/k/v for ALL 16384 tokens but only its 128 head-dims
(1/8 of the projection work), runs both attention passes for its heads,
and contracts its 256 rows of Wo (128 width + 128 height) into a
full-shape partial output [16384, 1024]. The host sums the 8 partials.

Phase structure (pipelined for engine balance):
  QW: per 512-token tile: stream hidden (fp8 hi/lo), QKV matmuls + RoPE,
      AND the width-attention for the tile's own 4 rows (width attention
      is row-local, so it fuses into the DMA-bound projection loop).
  H:  per 32-column chunk: SBUF->SBUF re-gather of v into column order,
      height attention -> catWH[...,1,:].
  OP: per 4-row group: out projection (f16, hf-merged [128,1024] psum
      tiles), single psum->SBUF copy per line, DMA out.

Layouts (natural head order: partition p = (head p//64, dim p%64)):
  qT/kT  [128, 128h, 128w] f16  d-major, token t = h*128+w on free dims
  vw     [128, 128t, 128d] f16  token-major (partition = w, tile = h)
  catWH  [128, 128h, 2, 128w] f16  attention outputs, both passes
  RoPE: rotate-half via +-1 permutation matmul (PI) on the PE, then
  qT = q*cos + (PI q)*sin on DVE/Pool.
  Attention (k-major probs): s=kT.T@qT -> exp on ACT (scale 1/32, one
  [128,2,512] op per group) -> l via one batched ones-matmul per head ->
  1/l on DVE -> AV matmul -> normalize on DVE.
"""

from contextlib import ExitStack

import numpy as np

import concourse.bass as bass
import concourse.mybir as mybir
import concourse.tile as tile
from concourse import bacc
from concourse.bass import ds, ts
from concourse.bass_utils import run_bass_kernel_spmd

F32 = mybir.dt.float32
F16 = mybir.dt.float16
F8 = mybir.dt.float8e4
DR = mybir.MatmulPerfMode.DoubleRow
EXP = mybir.ActivationFunctionType.Exp

N_CORES = 8
H_DIM = W_DIM = 128
D_MODEL = 1024
N_HEADS = 16
HEAD_DIM = 64
T_TOK = H_DIM * W_DIM          # 16384 tokens
N_QT = 32                      # QKV tiles of 512 tokens
SCALE = 1.0 / 32.0 / 256.0     # 1/sqrt(1024), W/Wo scaled 16x each


def build(reps: int = 1, stages: str = "all"):
    nc = bacc.Bacc("TRN2", target_bir_lowering=False, debug=False)

    h2_d = nc.dram_tensor("h2", [2, 8, 128, T_TOK], F8,
                          kind="ExternalInput").ap()
    cs_d = nc.dram_tensor("cs", [2, 128, T_TOK], F16, kind="ExternalInput").ap()
    wq_d = nc.dram_tensor("wq", [8, 128, 128], F8, kind="ExternalInput").ap()
    wk_d = nc.dram_tensor("wk", [8, 128, 128], F8, kind="ExternalInput").ap()
    wv_d = nc.dram_tensor("wv", [3, 8, 128, 128], F8, kind="ExternalInput").ap()
    wo_d = nc.dram_tensor("wo", [128, 2, D_MODEL], F16, kind="ExternalInput").ap()
    pi_d = nc.dram_tensor("pi", [128, 128], F16, kind="ExternalInput").ap()
    out_d = nc.dram_tensor("out", [H_DIM, 128, D_MODEL], F16,
                           kind="ExternalOutput").ap()
    vstg_d = nc.dram_tensor("vstg", [128, 128, 128], F16).ap()

    with tile.TileContext(nc) as tc, ExitStack() as ctx:
        pers = ctx.enter_context(tc.tile_pool(name="pers", bufs=1, space="SBUF"))

        qT = pers.tile([128, 128, 128], F16, name="qT")
        kT = pers.tile([128, 128, 128], F16, name="kT")
        vw = pers.tile([128, 128, 128], F16, name="vw")
        catWH = pers.tile([128, 128, 2, 128], F16, name="catWH")
        wq_sb = pers.tile([128, 8, 128], F8, name="wq_sb")
        wk_sb = pers.tile([128, 8, 128], F8, name="wk_sb")
        wv_sb = pers.tile([128, 3, 8, 128], F8, name="wv_sb")
        wo_sb = pers.tile([128, 2, D_MODEL], F16, name="wo_sb")
        pi_sb = pers.tile([128, 128], F16, name="pi_sb")
        ones_sb = pers.tile([128, 64], F16, name="ones_sb")

        nc.sync.dma_start(out=wv_sb, in_=wv_d.rearrange("s c p m -> p s c m"))
        nc.vector.memset(ones_sb, 1.0)

        for _rep in range(reps):
            # ------- Phase QW: QKV + RoPE + width attention (row-local) ----
            with ExitStack() as qctx:
                pht = qctx.enter_context(
                    tc.tile_pool(name="pht", bufs=2, space="SBUF"))
                pcs = qctx.enter_context(
                    tc.tile_pool(name="pcs", bufs=2, space="SBUF"))
                ptmp = qctx.enter_context(
                    tc.tile_pool(name="ptmp", bufs=1, space="SBUF"))
                pp16 = qctx.enter_context(
                    tc.tile_pool(name="qp16", bufs=2, space="SBUF"))
                pinv = qctx.enter_context(
                    tc.tile_pool(name="qinv", bufs=2, space="SBUF"))
                psqk = qctx.enter_context(
                    tc.tile_pool(name="psqk", bufs=2, space="PSUM"))
                psv = qctx.enter_context(
                    tc.tile_pool(name="psv", bufs=1, space="PSUM"))
                psrot = qctx.enter_context(
                    tc.tile_pool(name="psrot", bufs=1, space="PSUM"))
                pss = qctx.enter_context(
                    tc.tile_pool(name="qss", bufs=1, space="PSUM"))
                pslb = qctx.enter_context(
                    tc.tile_pool(name="qslb", bufs=1, space="PSUM"))
                pso = qctx.enter_context(
                    tc.tile_pool(name="qso", bufs=1, space="PSUM"))

                p16s = {}

                def w_scores_exp(u):
                    # scores + exp for rows 4u..4u+3 (consumes qT/kT tile u)
                    a0 = u * 4
                    ps_s = pss.tile([128, 2, 512], F32, tag="s", name="ps_s")
                    for j in range(4):
                        for h in range(2):
                            nc.tensor.matmul(
                                ps_s[:, h, ds(j * 128, 128)],
                                kT[ds(h * 64, 64), a0 + j, :],
                                qT[ds(h * 64, 64), a0 + j, :],
                                start=True, stop=True)
                    p16 = pp16.tile([128, 2, 512], F16, tag="p", name="p16")
                    nc.scalar.activation(p16[:], ps_s[:], EXP, scale=SCALE)
                    p16s[u] = p16

                def w_finish(u):
                    # lb + AV + normalize for rows 4u..4u+3
                    a0 = u * 4
                    p16 = p16s.pop(u)
                    ps_lb = pslb.tile([128, 512], F32, tag="lb", name="ps_lb")
                    ps_o = pso.tile([128, 512], F32, tag="o", name="ps_o")
                    for h in range(2):
                        nc.tensor.matmul(
                            ps_lb[ds(h * 64, 64), :],
                            ones_sb[:, 0:64], p16[:, h, :],
                            start=True, stop=True)
                    for j in range(4):
                        for h in range(2):
                            nc.tensor.matmul(
                                ps_o[ds(h * 64, 64), ds(j * 128, 128)],
                                vw[:, a0 + j, ds(h * 64, 64)],
                                p16[:, h, ds(j * 128, 128)],
                                start=True, stop=True)
                    inv = pinv.tile([128, 512], F32, tag="inv", name="inv")
                    nc.vector.reciprocal_approx_fast(inv[:], ps_lb[:])
                    nc.vector.tensor_mul(catWH[:, ds(a0, 4), 0, :],
                                         ps_o[:], inv[:])

                for t in range(N_QT):
                    h2 = pht.tile([128, 2, 8, 512], F8, tag="h2", name="h2")
                    nc.sync.dma_start(
                        out=h2,
                        in_=h2_d[:, :, :, ts(t, 512)].rearrange(
                            "s c p t -> p s c t"))
                    ht = h2[:, 0]
                    hlo = h2[:, 1]
                    cs = pcs.tile([128, 2, 512], F16, tag="cs", name="cs")
                    nc.sync.dma_start(
                        out=cs,
                        in_=cs_d[:, :, ts(t, 512)].rearrange("s p t -> p s t"))
                    if t == 0 and _rep == 0:
                        # emitted after the first hidden tile so the wv+h2
                        # transfers (needed first) win the DMA queue
                        nc.sync.dma_start(
                            out=wq_sb, in_=wq_d.rearrange("c p m -> p c m"))
                        nc.sync.dma_start(
                            out=wk_sb, in_=wk_d.rearrange("c p m -> p c m"))
                        nc.sync.dma_start(out=pi_sb, in_=pi_d)

                    def proj_qk(which, w_sb):
                        ps = psqk.tile([128, 512], F32, tag="qk",
                                       name="ps_qk")
                        for c in range(4):
                            nc.tensor.matmul(ps[:], w_sb[:, ds(2 * c, 2), :],
                                             ht[:, ds(2 * c, 2), :],
                                             start=(c == 0), stop=(c == 3),
                                             perf_mode=DR)
                        x16 = ptmp.tile([128, 512], F16, tag=f"x{which}",
                                        name="x16", bufs=2)
                        nc.scalar.copy(x16[:], ps[:])
                        return x16

                    def rope(which, x16, dstT):
                        ps_rot = psrot.tile([128, 512], F32,
                                            tag="rot", name="ps_rot")
                        nc.tensor.matmul(ps_rot[:], pi_sb[:], x16[:],
                                         start=True, stop=True)
                        xc = ptmp.tile([128, 512], F16, tag=f"c{which}",
                                       name="xc")
                        nc.vector.tensor_mul(xc[:], x16[:], cs[:, 0, :])
                        xs = ptmp.tile([128, 512], F16, tag=f"s{which}",
                                       name="xs")
                        nc.vector.tensor_mul(xs[:], ps_rot[:], cs[:, 1, :])
                        dview = dstT[:, ds(t * 4, 4), :]
                        nc.gpsimd.tensor_add(dview, xc[:], xs[:])

                    # q proj first so its psum->SBUF copy overlaps v matmuls
                    x16q = proj_qk(0, wq_sb)

                    # v (token-major): 4 sub-tiles of 128 tokens
                    ps_v = psv.tile([128, 512], F32, tag="v", name="ps_v")
                    for sub in range(4):
                        for si, (hsrc, wi) in enumerate(
                                ((ht, 0), (hlo, 1), (ht, 2))):
                            for c in range(4):
                                nc.tensor.matmul(
                                    ps_v[:, ds(sub * 128, 128)],
                                    hsrc[:, ds(2 * c, 2), ds(sub * 128, 128)],
                                    wv_sb[:, wi, ds(2 * c, 2), :],
                                    start=(si == 0 and c == 0),
                                    stop=(si == 2 and c == 3),
                                    perf_mode=DR)

                    rope(0, x16q, qT)
                    x16k = proj_qk(1, wk_sb)

                    if stages != "q":
                        # width attention, software-pipelined behind the
                        # projections so the PE never waits on exp (ACT)
                        if t >= 1:
                            w_scores_exp(t - 1)
                        if t >= 2:
                            w_finish(t - 2)

                    rope(1, x16k, kT)
                    vw_view = vw[:, ds(t * 4, 4), :]
                    nc.scalar.copy(vw_view, ps_v[:])
                    nc.sync.dma_start(out=vstg_d[:, ds(t * 4, 4), :],
                                      in_=vw_view)

                if stages != "q":
                    w_scores_exp(N_QT - 1)
                    w_finish(N_QT - 2)
                    w_finish(N_QT - 1)

            if stages == "q":
                continue
            # wo is only needed by phase OP; loading it here keeps the
            # startup DMAs small
            nc.sync.dma_start(out=wo_sb, in_=wo_d)
            # ---------------- Phase H: height attention ----------------
            # column c: tokens t = h*128 + c; scores contract over full qT/kT
            with ExitStack() as hctx:
                pvh = hctx.enter_context(
                    tc.tile_pool(name="pvh", bufs=2, space="SBUF"))
                pp16 = hctx.enter_context(
                    tc.tile_pool(name="pp16", bufs=3, space="SBUF"))
                pinv = hctx.enter_context(
                    tc.tile_pool(name="pinv", bufs=2, space="SBUF"))
                pss = hctx.enter_context(
                    tc.tile_pool(name="pss", bufs=2, space="PSUM"))
                pslb = hctx.enter_context(
                    tc.tile_pool(name="pslb", bufs=2, space="PSUM"))
                pso = hctx.enter_context(
                    tc.tile_pool(name="pso", bufs=2, space="PSUM"))

                vhs = {}
                hp16 = {}

                def h_load(cg):
                    vh = pvh.tile([128, 32, 128], F16, tag="vh", name="vh")
                    nc.sync.dma_start(
                        out=vh,
                        in_=vstg_d[ds(cg * 32, 32), :, :].rearrange(
                            "c t d -> t c d"))
                    vhs[cg] = vh

                def h_scores_exp(u):
                    c0 = u * 4
                    ps_s = pss.tile([128, 2, 512], F32, tag="s", name="ps_s")
                    for j in range(4):
                        for h in range(2):
                            nc.tensor.matmul(
                                ps_s[:, h, ds(j * 128, 128)],
                                kT[ds(h * 64, 64), :, c0 + j],
                                qT[ds(h * 64, 64), :, c0 + j],
                                start=True, stop=True)
                    p16 = pp16.tile([128, 2, 512], F16, tag="p", name="p16")
                    nc.scalar.activation(p16[:], ps_s[:], EXP, scale=SCALE)
                    hp16[u] = p16

                def h_finish(u):
                    c0 = u * 4
                    vh = vhs[u // 8]
                    p16 = hp16.pop(u)
                    ps_lb = pslb.tile([128, 512], F32, tag="lb", name="ps_lb")
                    ps_o = pso.tile([128, 512], F32, tag="o", name="ps_o")
                    for h in range(2):
                        nc.tensor.matmul(
                            ps_lb[ds(h * 64, 64), :],
                            ones_sb[:, 0:64], p16[:, h, :],
                            start=True, stop=True)
                    for j in range(4):
                        for h in range(2):
                            nc.tensor.matmul(
                                ps_o[ds(h * 64, 64), ds(j * 128, 128)],
                                vh[:, (u % 8) * 4 + j, ds(h * 64, 64)],
                                p16[:, h, ds(j * 128, 128)],
                                start=True, stop=True)
                    inv = pinv.tile([128, 512], F32, tag="inv", name="inv")
                    nc.vector.reciprocal_approx_fast(inv[:], ps_lb[:])
                    dst = catWH[:, :, 1, ds(c0, 4)].rearrange("p h c -> p c h")
                    nc.vector.tensor_mul(dst, ps_o[:], inv[:])

                h_load(0)
                h_scores_exp(0)
                h_scores_exp(1)
                for u in range(32):          # groups of 4 columns
                    if u % 8 == 0 and u // 8 + 1 < 4:
                        h_load(u // 8 + 1)
                    h_finish(u)
                    if u + 2 < 32:
                        h_scores_exp(u + 2)

            # ---------------- Phase OP: out projection ----------------
            with ExitStack() as wctx:
                pout = wctx.enter_context(
                    tc.tile_pool(name="pout", bufs=3, space="SBUF"))
                psop = wctx.enter_context(
                    tc.tile_pool(name="wsop", bufs=4, space="PSUM"))

                for g in range(32):          # groups of 4 lines
                    a0 = g * 4
                    out16 = pout.tile([128, 4, D_MODEL], F16, tag="ob",
                                      name="out16")
                    for j in range(4):
                        for hf in range(2):
                            ps_op = psop.tile([128, 512], F32, tag="op",
                                              name="ps_op")
                            nc.tensor.matmul(ps_op[:],
                                             catWH[:, a0 + j, 0, :],
                                             wo_sb[:, 0, ds(hf * 512, 512)],
                                             start=True, stop=False)
                            nc.tensor.matmul(ps_op[:],
                                             catWH[:, a0 + j, 1, :],
                                             wo_sb[:, 1, ds(hf * 512, 512)],
                                             start=False, stop=True)
                            dst = out16[:, j, ds(hf * 512, 512)]
                            if hf == 0:
                                nc.scalar.copy(dst, ps_op[:])
                            else:
                                nc.vector.tensor_copy(dst, ps_op[:])
                    nc.sync.dma_start(
                        out=out_d[ds(a0, 4)].rearrange("l p m -> p l m"),
                        in_=out16)

    nc.compile()
    return nc


_NC_CACHE = {}


def _get_nc(reps: int = 1, stages: str = "all"):
    key = (reps, stages)
    if key not in _NC_CACHE:
        _NC_CACHE[key] = build(reps, stages)
    return _NC_CACHE[key]


def prep_in_maps(hidden_state, cos, sin, Wq, Wk, Wv, Wo):
    import ml_dtypes
    F8NP = ml_dtypes.float8_e4m3fn
    hid = np.asarray(hidden_state, np.float32).reshape(T_TOK, D_MODEL)
    hTf = np.ascontiguousarray(hid.T)
    hHI = hTf.astype(F8NP)
    hLO = ((hTf - hHI.astype(np.float32)) * 16.0).astype(F8NP)
    h2 = np.ascontiguousarray(
        np.stack([hHI, hLO]).reshape(2, 8, 128, T_TOK))
    cosr = np.asarray(cos, np.float32).reshape(T_TOK, HEAD_DIM)
    sinr = np.asarray(sin, np.float32).reshape(T_TOK, HEAD_DIM)
    cs = np.stack([
        np.vstack([cosr.T, cosr.T]),
        np.vstack([sinr.T, sinr.T]),
    ]).astype(np.float16)
    pi = np.zeros((128, 128), np.float16)
    for h in range(2):
        for dp in range(32):
            pi[h * 64 + dp + 32, h * 64 + dp] = -1.0
            pi[h * 64 + dp, h * 64 + dp + 32] = 1.0
    Wq_ = np.asarray(Wq, np.float32)
    Wk_ = np.asarray(Wk, np.float32)
    Wv_ = np.asarray(Wv, np.float32)
    Wo_ = np.asarray(Wo, np.float32)

    in_maps = []
    for i in range(N_CORES):
        sl = slice(128 * i, 128 * i + 128)
        m = {
            "h2": h2,
            "cs": cs,
            "pi": pi,
            "wq": np.ascontiguousarray(
                Wq_[:, sl].reshape(8, 128, 128) * 16.0).astype(F8NP),
            "wk": np.ascontiguousarray(
                Wk_[:, sl].reshape(8, 128, 128) * 16.0).astype(F8NP),
            "wv": _wv_triple(Wv_[:, sl] * 16.0, F8NP),
            "wo": np.ascontiguousarray(
                np.stack([Wo_[sl, :], Wo_[D_MODEL + 128 * i:
                                          D_MODEL + 128 * i + 128, :]],
                         axis=1)).astype(np.float16),
        }
        in_maps.append(m)
    return in_maps


def _wv_triple(Wvp, F8NP):
    A = Wvp.astype(F8NP)
    B = (Wvp / 16.0).astype(F8NP)
    C = (Wvp - A.astype(np.float32)).astype(F8NP)
    out = np.stack([A, B, C]).reshape(3, 8, 128, 128)
    return np.ascontiguousarray(out)


def assemble(results):
    acc = np.zeros((T_TOK, D_MODEL), dtype=np.float32)
    for r in results:
        acc += r["out"].reshape(T_TOK, D_MODEL).astype(np.float32)
    return (acc / 16.0).reshape(H_DIM, W_DIM, D_MODEL)


def kernel(hidden_state, attn_mask, cos, sin, Wq, Wk, Wv, Wo):
    nc = _get_nc(1)
    in_maps = prep_in_maps(hidden_state, cos, sin, Wq, Wk, Wv, Wo)
    res = run_bass_kernel_spmd(nc, in_maps, list(range(N_CORES)))
    return assemble(res.results)
